# revision 53
# baseline (speedup 1.0000x reference)
"""Trainium2 Bass kernel for nn_BasicBlock (binary activation + binarized
weight-standardized 3x3 conv + residual + PReLU).

Contract: kernel(**inputs) takes FULL unsharded numpy inputs (keys as in
setup_inputs) and returns the FULL [32, 512, 28, 28] float32 output.
Internally shards the batch dim across 8 NeuronCores (4 images each); the
small conv weight + per-channel vectors are replicated.

Two program variants, selected at runtime from the actual input values:
- specialized (move*_bias == 0, gain == 1): 2-op epilogue per tile:
  z = alphabar*acc + residual on DVE, then a single ACT Prelu with
  per-channel alpha; both half-tiles of an image share one batched
  output DMA.
- general: 4-op epilogue applying the move biases and gain exactly.

Shared structure:
- fp8e4 DoubleRow conv, 18 accumulation rounds per [128, 420] PSUM tile;
  2 tiles per PSUM group so a weight load serves back-to-back matmuls and
  the issue rate stays at the pure-stream floor (~177 ns / 420-pixel MM).
- weight prep per cout chunk: piecewise bn_stats -> sign (ACT) -> PE
  transposes (bf16) -> DVE fp8 casts, pipelined at (tap, pair)
  granularity and overlapped with the previous chunk's conv.
- DMA issue order == hw-queue service order, laid out by need time;
  junk transposes warm the PE clock gate before the first real conv.
"""

import numpy as np

import concourse.bass as bass
import concourse.mybir as mybir
import concourse.tile as tile
from concourse import bacc
from concourse.masks import make_identity

# problem constants (hardcoded per harness contract)
N_CORES = 8
N_PER = 4          # images per core (32 / 8)
C = 512            # Cin == Cout
H = W = 28
HP = WP = 30       # zero-padded spatial
TAPS = 9
KFAN = C * TAPS    # 4608 = fan-in per output channel
ALPHA = 0.2
BETA = 1.0
EPS = 1e-5
WS_SCALE = 1.0 / float(np.sqrt(KFAN))  # fan_in**-0.5
NCH = C // 128     # 4 channel chunks of 128
NPAIR = NCH // 2   # 2 DoubleRow pairs of chunks
ROWS_PER_TILE = 14 # output rows per matmul tile
NSPAT = H // ROWS_PER_TILE  # 2 spatial tiles per image
NFREE = ROWS_PER_TILE * WP  # 420: contiguous run incl. 2 pad cols per row
ACT_IMG = 912  # padded 30x30 image (900) + 12 slack: %16==0 for DoubleRow,
               # and covers the last tile's 420-run overhang (482+420=902)
NROUND = NPAIR * TAPS  # 18 accumulation rounds per output tile

FP32 = mybir.dt.float32
BF16 = mybir.dt.bfloat16
FP8 = mybir.dt.float8e4


def build_program(specialized):
    nc = bacc.Bacc(
        "TRN2",
        target_bir_lowering=False,
        debug=False,
        num_devices=1,
        num_swdge_queues=1,
    )
    x_h = nc.declare_dram_parameter("x", [N_PER, C, H, W], FP32, isOutput=False)
    w_h = nc.declare_dram_parameter("conv_weight", [C, C, 3, 3], FP32, isOutput=False)
    gain_h = nc.declare_dram_parameter("gain", [C], FP32, isOutput=False)
    b0_h = nc.declare_dram_parameter("move0_bias", [C], FP32, isOutput=False)
    b1_h = nc.declare_dram_parameter("move1_bias", [C], FP32, isOutput=False)
    pa_h = nc.declare_dram_parameter("prelu_a", [C], FP32, isOutput=False)
    b2_h = nc.declare_dram_parameter("move2_bias", [C], FP32, isOutput=False)
    out_h = nc.declare_dram_parameter("out", [N_PER, C, H, W], FP32, isOutput=True)

    x_ap = x_h[:, :, :, :]
    w_ap = w_h[:, :, :, :]
    out_ap = out_h[:, :, :, :]

    with tile.TileContext(nc) as tc:
        with (
            tc.tile_pool(name="persist", bufs=1) as persist,
            tc.tile_pool(name="scratch", bufs=2) as scratch,
            tc.tile_pool(name="stats", bufs=4) as stats,
            tc.tile_pool(name="epi", bufs=4) as epi,
            tc.tile_pool(name="psum_mm", bufs=6, space="PSUM") as psum_mm,
            tc.tile_pool(name="psum_tr", bufs=2, space="PSUM") as psum_tr,
        ):
            # ---- identity for PE transposes + HAM warm-up ---------------
            ident = persist.tile([128, 128], BF16, tag="ident")
            make_identity(nc, ident)

            # ---- small per-channel vectors: tiny hwdge DMAs up front ----
            def load_vec(eng, h, nm):
                t = persist.tile([128, NCH], FP32, tag=f"v_{nm}", name=f"v_{nm}")
                eng.dma_start(
                    out=t, in_=h[:].rearrange("(c p) -> p c", p=128)
                )
                return t

            pa_v = load_vec(nc.scalar, pa_h, "pa")
            pa_c = [pa_v[:, c : c + 1] for c in range(NCH)]
            if not specialized:
                b0_v = load_vec(nc.sync, b0_h, "b0")
                gain_v = load_vec(nc.sync, gain_h, "gain")
                b1_v = load_vec(nc.scalar, b1_h, "b1")
                b2_v = load_vec(nc.scalar, b2_h, "b2")
                gain_c = [gain_v[:, c : c + 1] for c in range(NCH)]
                b0_c = [b0_v[:, c : c + 1] for c in range(NCH)]
                b1_c = [b1_v[:, c : c + 1] for c in range(NCH)]
                b2_c = [b2_v[:, c : c + 1] for c in range(NCH)]

            # ---- activation image tiles; image 0/1 memsets first --------
            act_img = [[None] * N_PER for _ in range(NPAIR)]
            for n in range(N_PER):
                for q in range(NPAIR):
                    act_img[q][n] = persist.tile(
                        [128, 2, ACT_IMG], FP8, tag=f"act{q}_{n}", name=f"act{q}_{n}"
                    )
            for n in (0, 1):
                for q in range(NPAIR):
                    nc.gpsimd.memset(act_img[q][n], 0.0)

            # ---- HAM warm-up: junk transposes keep PE busy pre-conv ------
            junk_ps = psum_tr.tile([128, 2, 128], BF16, tag="ptr", name="junk")
            for _ in range(120):
                nc.tensor.transpose(junk_ps[:, 0, :], ident, ident)

            # ---- weight + x DMAs in hw-queue need order ------------------
            w_flat = w_ap.rearrange("o i a b -> o (i a b)")
            w_tiles = [None] * NCH
            xs_tiles = [
                persist.tile([128, N_PER, H, W], FP32, tag=f"xs{c}", name=f"xs{c}")
                for c in range(NCH)
            ]
            xr = x_ap.rearrange("n c h w -> c n h w")

            def x_dma(eng, c, n0, n1):
                eng.dma_start(
                    out=xs_tiles[c][:, n0:n1],
                    in_=xr[c * 128 : (c + 1) * 128, n0:n1],
                )

            def w_dma_chunk(m):
                w_tiles[m] = scratch.tile(
                    [128, KFAN], FP32, tag="wtile", name=f"wt{m}"
                )
                nc.sync.dma_start(
                    out=w_tiles[m],
                    in_=w_flat[m * 128 : (m + 1) * 128, :],
                )

            w_tiles[0] = scratch.tile([128, KFAN], FP32, tag="wtile", name="wt0")
            for j in range(TAPS):
                eng = nc.sync if j % 2 == 0 else nc.scalar
                eng.dma_start(
                    out=w_tiles[0][:, j * 512 : (j + 1) * 512],
                    in_=w_flat[0:128, j * 512 : (j + 1) * 512],
                )
            x_dma(nc.sync, 0, 0, 1)
            x_dma(nc.sync, 1, 0, 1)
            x_dma(nc.sync, 2, 0, 2)   # pair-1 acts for images 0-1
            x_dma(nc.sync, 3, 0, 2)
            x_dma(nc.sync, 0, 1, 2)   # pair-0 acts for image 1
            x_dma(nc.sync, 1, 1, 2)
            x_dma(nc.sync, 0, 2, 4)   # image 2-3 pair-0
            x_dma(nc.sync, 1, 2, 4)
            w_dma_chunk(1)
            x_dma(nc.sync, 2, 2, 4)   # image 2-3 pair-1
            x_dma(nc.sync, 3, 2, 4)
            w_dma_chunk(2)
            w_dma_chunk(3)

            # ---- act sign ------------------------------------------------
            def xsign(n, c):
                dst = act_img[c // 2][n][:, c % 2, : HP * WP].rearrange(
                    "p (h w) -> p h w", w=WP
                )[:, 1 : 1 + H, 1 : 1 + W]
                nc.scalar.activation(
                    out=dst,
                    in_=xs_tiles[c][:, n],
                    func=mybir.ActivationFunctionType.Sign,
                    bias=0.0 if specialized else b0_c[c],
                    scale=BETA,
                )

            # lhsT : [128(cin), tap, pair, half, cout] fp8 DoubleRow weights
            lhsT = persist.tile(
                [128, TAPS, NPAIR, 2, C], FP8, tag="lhsT", name="lhsT"
            )
            alphabar = {}   # per cout chunk [128,1]: (2*)alpha*sf*gain
            wsigns = {}
            mvs = {}

            def weight_prep_a(m):
                """stats -> negmean: the critical path to the signs"""
                wt = w_tiles[m]
                st = stats.tile([128, TAPS, 6], FP32, tag="bnst", name="bnst")
                wt3 = wt.rearrange("p (a b) -> p a b", b=512)
                for sg in range(TAPS):
                    nc.vector.bn_stats(out=st[:, sg, :], in_=wt3[:, sg, :])
                mv = stats.tile([128, 2], FP32, tag="bnagg", name="bnagg")
                nc.vector.bn_aggr(out=mv, in_=st)

                negmean = stats.tile([128, 1], FP32, tag="negmean", name="negmean")
                nc.vector.tensor_scalar_mul(out=negmean, in0=mv[:, 0:1], scalar1=-1.0)
                mvs[m] = (mv, negmean)

            def weight_sign(m, q):
                """sign(w - mean) -> bf16 (+-1) for pair q's two cin blocks"""
                wt = w_tiles[m]
                _, negmean = mvs[m]
                if q == 0:
                    ws = scratch.tile([128, KFAN], BF16, tag="wsign", name="wsign")
                    wsigns[m] = ws
                ws = wsigns[m]
                for h2 in range(2):
                    b = 2 * q + h2
                    nc.scalar.activation(
                        out=ws[:, b * 1152 : (b + 1) * 1152],
                        in_=wt[:, b * 1152 : (b + 1) * 1152],
                        func=mybir.ActivationFunctionType.Sign,
                        bias=negmean,
                    )

            def weight_transpose(m, q):
                """per tap: 2 block transposes -> 1 fp8 cast (pair q)"""
                ws3 = wsigns[m].rearrange("p (i t) -> p i t", t=TAPS)
                for t in range(TAPS):
                    ps = psum_tr.tile(
                        [128, 2, 128], BF16, tag="ptr", name="ptr"
                    )
                    for h2 in range(2):
                        b = 2 * q + h2
                        nc.tensor.transpose(
                            ps[:, h2, :],
                            ws3[:, b * 128 : (b + 1) * 128, t],
                            ident,
                        )
                    nc.vector.tensor_copy(
                        out=lhsT[:, t, q, :, m * 128 : (m + 1) * 128],
                        in_=ps,
                    )

            def weight_prep_c(m):
                """1/(std+eps), sum|w-mean| -> alphabar; off critical path"""
                wt = w_tiles[m]
                mv, negmean = mvs[m]
                stdeps = stats.tile([128, 1], FP32, tag="stdeps", name="stdeps")
                nc.scalar.activation(
                    out=stdeps, in_=mv[:, 1:2], func=mybir.ActivationFunctionType.Sqrt
                )
                nc.vector.tensor_scalar_add(out=stdeps, in0=stdeps, scalar1=EPS)
                inv = stats.tile([128, 1], FP32, tag="inv", name="inv")
                nc.vector.reciprocal(out=inv, in_=stdeps)

                sumabs = stats.tile([128, NCH], FP32, tag="sumabs", name="sumabs")
                for b in range(NCH):
                    nc.scalar.activation(
                        out=wt[:, b * 1152 : (b + 1) * 1152],
                        in_=wt[:, b * 1152 : (b + 1) * 1152],
                        func=mybir.ActivationFunctionType.Abs,
                        bias=negmean,
                        accum_out=sumabs[:, b : b + 1],
                    )
                sumabs1 = stats.tile([128, 1], FP32, tag="sumabs1", name="sumabs1")
                nc.vector.tensor_reduce(
                    out=sumabs1, in_=sumabs, axis=mybir.AxisListType.X,
                    op=mybir.AluOpType.add,
                )

                ab = persist.tile(
                    [128, 1], FP32, tag=f"alphabar{m}", name=f"alphabar{m}"
                )
                nc.vector.tensor_tensor(
                    out=ab, in0=sumabs1, in1=inv, op=mybir.AluOpType.mult
                )
                if specialized:
                    # gain == 1
                    nc.vector.tensor_scalar_mul(
                        out=ab, in0=ab, scalar1=ALPHA * WS_SCALE / KFAN
                    )
                else:
                    nc.vector.tensor_tensor(
                        out=ab, in0=ab, in1=gain_c[m], op=mybir.AluOpType.mult
                    )
                    nc.vector.tensor_scalar_mul(
                        out=ab, in0=ab, scalar1=ALPHA * WS_SCALE / KFAN
                    )
                alphabar[m] = ab

            def epilogue(m, n, h2, acc, o=None):
                y0 = h2 * ROWS_PER_TILE
                accv = acc.rearrange("p (h w) -> p h w", w=WP)[:, :, 0:W]
                res = xs_tiles[m][:, n, y0 : y0 + ROWS_PER_TILE, :]
                if specialized:
                    # z = acc*alphabar + residual ; out = prelu(z); biases 0
                    z = epi.tile(
                        [128, ROWS_PER_TILE, W], FP32, tag="z", name="z"
                    )
                    nc.vector.scalar_tensor_tensor(
                        out=z, in0=accv, scalar=alphabar[m], in1=res,
                        op0=mybir.AluOpType.mult, op1=mybir.AluOpType.add,
                    )
                    nc.scalar.activation(
                        out=o[:, y0 : y0 + ROWS_PER_TILE, :], in_=z,
                        func=mybir.ActivationFunctionType.Prelu,
                        alpha=pa_c[m],
                    )
                    if h2 == NSPAT - 1:
                        # one batched store for the whole (chunk, image)
                        nc.sync.dma_start(
                            out=out_ap[n, m * 128 : (m + 1) * 128, :, :],
                            in_=o,
                        )
                    return
                # z = acc*alphabar + residual   (prelu input minus b1)
                z = epi.tile(
                    [128, ROWS_PER_TILE, W], FP32, tag="z", name="z"
                )
                nc.vector.scalar_tensor_tensor(
                    out=z, in0=accv, scalar=alphabar[m], in1=res,
                    op0=mybir.AluOpType.mult, op1=mybir.AluOpType.add,
                )
                # r = relu(z + b1) on ACT
                r = epi.tile(
                    [128, ROWS_PER_TILE, W], FP32, tag="r", name="r"
                )
                nc.scalar.activation(
                    out=r, in_=z,
                    func=mybir.ActivationFunctionType.Relu,
                    bias=b1_c[m],
                )
                # zz = a*z + (a*b1 + b2) ; out = (1-a)*r + zz
                zz = epi.tile(
                    [128, ROWS_PER_TILE, W], FP32, tag="zz", name="zz"
                )
                nc.scalar.activation(
                    out=zz, in_=z,
                    func=mybir.ActivationFunctionType.Identity,
                    scale=pa_c[m], bias=ab1b2[m],
                )
                nc.vector.scalar_tensor_tensor(
                    out=zz, in0=r, scalar=one_minus_a[m], in1=zz,
                    op0=mybir.AluOpType.mult, op1=mybir.AluOpType.add,
                )
                nc.sync.dma_start(
                    out=out_ap[
                        n, m * 128 : (m + 1) * 128,
                        y0 : y0 + ROWS_PER_TILE, :,
                    ],
                    in_=zz,
                )

            def conv_rounds(m, group, accs, q):
                """emit the 9 accumulation rounds of pair q for a group"""
                for t in range(TAPS):
                    dy, dx = t // 3, t % 3
                    wslice = lhsT[:, t, q, :, m * 128 : (m + 1) * 128]
                    for j, (n, h2) in enumerate(group):
                        base = (h2 * ROWS_PER_TILE + dy) * WP + dx
                        rhs = act_img[q][n][:, :, base : base + NFREE]
                        nc.tensor.matmul(
                            accs[j],
                            wslice,
                            rhs,
                            start=(q == 0 and t == 0),
                            stop=(q == NPAIR - 1 and t == TAPS - 1),
                            perf_mode=mybir.MatmulPerfMode.DoubleRow,
                        )

            def conv_open(m, group):
                return [
                    psum_mm.tile([128, NFREE], FP32, tag="acc", name="acc")
                    for _ in group
                ]

            def group_out_tile():
                if not specialized:
                    return None
                return epi.tile([128, H, W], FP32, tag="o", name="o")

            def conv_group(m, group):
                accs = conv_open(m, group)
                for q in range(NPAIR):
                    conv_rounds(m, group, accs, q)
                o = group_out_tile()
                for j, (n, h2) in enumerate(group):
                    epilogue(m, n, h2, accs[j], o)

            # ---- chunk 0 startup pipeline --------------------------------
            weight_prep_a(0)
            weight_sign(0, 0)          # blocks 0,1
            weight_transpose(0, 0)     # taps for pair 0
            xsign(0, 0)
            xsign(0, 1)
            weight_sign(0, 1)          # blocks 2,3

            if not specialized:
                # derived per-channel epilogue constants (emitted after the
                # startup-critical DVE work so they can't HOL-block the FIFO)
                oma_v = persist.tile([128, NCH], FP32, tag="oma")
                nc.vector.tensor_scalar(
                    out=oma_v, in0=pa_v, scalar1=-1.0, scalar2=1.0,
                    op0=mybir.AluOpType.mult, op1=mybir.AluOpType.add,
                )
                one_minus_a = [oma_v[:, c : c + 1] for c in range(NCH)]
                ab_v = persist.tile([128, NCH], FP32, tag="ab1b2")
                for c in range(NCH):
                    nc.vector.scalar_tensor_tensor(
                        out=ab_v[:, c : c + 1], in0=b1_c[c], scalar=pa_c[c],
                        in1=b2_c[c],
                        op0=mybir.AluOpType.mult, op1=mybir.AluOpType.add,
                    )
                ab1b2 = [ab_v[:, c : c + 1] for c in range(NCH)]

            g0 = [(0, 0), (0, 1)]
            g0_accs = conv_open(0, g0)
            conv_rounds(0, g0, g0_accs, 0)
            xsign(0, 2)
            xsign(0, 3)
            xsign(1, 0)
            xsign(1, 1)
            weight_transpose(0, 1)     # taps for pair 1
            weight_prep_c(0)
            conv_rounds(0, g0, g0_accs, 1)
            g0_o = group_out_tile()
            for j, (n, h2) in enumerate(g0):
                epilogue(0, n, h2, g0_accs[j], g0_o)
            xsign(1, 2)
            xsign(1, 3)
            conv_group(0, [(1, 0), (1, 1)])
            for q in range(NPAIR):
                nc.gpsimd.memset(act_img[q][2], 0.0)
            for c in range(NCH):
                xsign(2, c)
            conv_group(0, [(2, 0), (2, 1)])
            for q in range(NPAIR):
                nc.gpsimd.memset(act_img[q][3], 0.0)
            for c in range(NCH):
                xsign(3, c)
            weight_prep_a(1)
            weight_sign(1, 0)
            weight_sign(1, 1)
            weight_transpose(1, 0)
            weight_transpose(1, 1)
            conv_group(0, [(3, 0), (3, 1)])
            weight_prep_c(1)

            # ---- main loop: conv(m) interleaved with prep(m+1) -----------
            for m in range(1, NCH):
                conv_group(m, [(0, 0), (0, 1)])
                if m + 1 < NCH:
                    weight_prep_a(m + 1)
                    weight_sign(m + 1, 0)
                    weight_sign(m + 1, 1)
                conv_group(m, [(1, 0), (1, 1)])
                if m + 1 < NCH:
                    weight_transpose(m + 1, 0)
                conv_group(m, [(2, 0), (2, 1)])
                if m + 1 < NCH:
                    weight_transpose(m + 1, 1)
                conv_group(m, [(3, 0), (3, 1)])
                if m + 1 < NCH:
                    weight_prep_c(m + 1)

    nc.finalize()
    return nc


_NC_CACHE = {}


def _get_program(specialized=True):
    if specialized not in _NC_CACHE:
        _NC_CACHE[specialized] = build_program(specialized)
    return _NC_CACHE[specialized]


def kernel(**inputs):
    from concourse.bass_utils import run_bass_kernel_spmd

    x = np.ascontiguousarray(np.asarray(inputs["x"], dtype=np.float32))
    shared = {
        name: np.ascontiguousarray(np.asarray(inputs[name], dtype=np.float32))
        for name in (
            "conv_weight", "gain", "move0_bias", "move1_bias", "prelu_a",
            "move2_bias",
        )
    }
    specialized = bool(
        np.all(shared["move0_bias"] == 0.0)
        and np.all(shared["move1_bias"] == 0.0)
        and np.all(shared["move2_bias"] == 0.0)
        and np.all(shared["gain"] == 1.0)
    )
    nc = _get_program(specialized)
    in_maps = [
        {"x": x[i * N_PER : (i + 1) * N_PER], **shared} for i in range(N_CORES)
    ]
    res = run_bass_kernel_spmd(nc, in_maps, core_ids=list(range(N_CORES)))
    return np.concatenate([r["out"] for r in res.results], axis=0)


# revision 54
# speedup vs baseline: 1.0016x; 1.0016x over previous
"""Trainium2 Bass kernel for nn_BasicBlock (binary activation + binarized
weight-standardized 3x3 conv + residual + PReLU).

Contract: kernel(**inputs) takes FULL unsharded numpy inputs (keys as in
setup_inputs) and returns the FULL [32, 512, 28, 28] float32 output.
Internally shards the batch dim across 8 NeuronCores (4 images each); the
small conv weight + per-channel vectors are replicated.

Two program variants, selected at runtime from the actual input values:
- specialized (move*_bias == 0, gain == 1): 2-op epilogue per tile:
  z = alphabar*acc + residual on DVE, then a single ACT Prelu with
  per-channel alpha; both half-tiles of an image share one batched
  output DMA.
- general: 4-op epilogue applying the move biases and gain exactly.

Shared structure:
- fp8e4 DoubleRow conv, 18 accumulation rounds per [128, 420] PSUM tile;
  2 tiles per PSUM group so a weight load serves back-to-back matmuls and
  the issue rate stays at the pure-stream floor (~177 ns / 420-pixel MM).
- weight prep per cout chunk: piecewise bn_stats -> sign (ACT) -> PE
  transposes (bf16) -> DVE fp8 casts, pipelined at (tap, pair)
  granularity and overlapped with the previous chunk's conv.
- DMA issue order == hw-queue service order, laid out by need time;
  junk transposes warm the PE clock gate before the first real conv.
"""

import numpy as np

import concourse.bass as bass
import concourse.mybir as mybir
import concourse.tile as tile
from concourse import bacc
from concourse.masks import make_identity

# problem constants (hardcoded per harness contract)
N_CORES = 8
N_PER = 4          # images per core (32 / 8)
C = 512            # Cin == Cout
H = W = 28
HP = WP = 30       # zero-padded spatial
TAPS = 9
KFAN = C * TAPS    # 4608 = fan-in per output channel
ALPHA = 0.2
BETA = 1.0
EPS = 1e-5
WS_SCALE = 1.0 / float(np.sqrt(KFAN))  # fan_in**-0.5
NCH = C // 128     # 4 channel chunks of 128
NPAIR = NCH // 2   # 2 DoubleRow pairs of chunks
ROWS_PER_TILE = 14 # output rows per matmul tile
NSPAT = H // ROWS_PER_TILE  # 2 spatial tiles per image
NFREE = ROWS_PER_TILE * WP  # 420: contiguous run incl. 2 pad cols per row
ACT_IMG = 912  # padded 30x30 image (900) + 12 slack: %16==0 for DoubleRow,
               # and covers the last tile's 420-run overhang (482+420=902)
NROUND = NPAIR * TAPS  # 18 accumulation rounds per output tile

FP32 = mybir.dt.float32
BF16 = mybir.dt.bfloat16
FP8 = mybir.dt.float8e4


def build_program(specialized):
    nc = bacc.Bacc(
        "TRN2",
        target_bir_lowering=False,
        debug=False,
        num_devices=1,
        num_swdge_queues=1,
    )
    x_h = nc.declare_dram_parameter("x", [N_PER, C, H, W], FP32, isOutput=False)
    w_h = nc.declare_dram_parameter("conv_weight", [C, C, 3, 3], FP32, isOutput=False)
    gain_h = nc.declare_dram_parameter("gain", [C], FP32, isOutput=False)
    b0_h = nc.declare_dram_parameter("move0_bias", [C], FP32, isOutput=False)
    b1_h = nc.declare_dram_parameter("move1_bias", [C], FP32, isOutput=False)
    pa_h = nc.declare_dram_parameter("prelu_a", [C], FP32, isOutput=False)
    b2_h = nc.declare_dram_parameter("move2_bias", [C], FP32, isOutput=False)
    out_h = nc.declare_dram_parameter("out", [N_PER, C, H, W], FP32, isOutput=True)

    x_ap = x_h[:, :, :, :]
    w_ap = w_h[:, :, :, :]
    out_ap = out_h[:, :, :, :]

    with tile.TileContext(nc) as tc:
        with (
            tc.tile_pool(name="persist", bufs=1) as persist,
            tc.tile_pool(name="scratch", bufs=2) as scratch,
            tc.tile_pool(name="stats", bufs=4) as stats,
            tc.tile_pool(name="epi", bufs=4) as epi,
            tc.tile_pool(name="psum_mm", bufs=6, space="PSUM") as psum_mm,
            tc.tile_pool(name="psum_tr", bufs=2, space="PSUM") as psum_tr,
        ):
            # ---- identity for PE transposes + HAM warm-up ---------------
            ident = persist.tile([128, 128], BF16, tag="ident")
            make_identity(nc, ident)

            # ---- small per-channel vectors: tiny hwdge DMAs up front ----
            def load_vec(eng, h, nm):
                t = persist.tile([128, NCH], FP32, tag=f"v_{nm}", name=f"v_{nm}")
                eng.dma_start(
                    out=t, in_=h[:].rearrange("(c p) -> p c", p=128)
                )
                return t

            pa_v = load_vec(nc.scalar, pa_h, "pa")
            pa_c = [pa_v[:, c : c + 1] for c in range(NCH)]
            if not specialized:
                b0_v = load_vec(nc.sync, b0_h, "b0")
                gain_v = load_vec(nc.sync, gain_h, "gain")
                b1_v = load_vec(nc.scalar, b1_h, "b1")
                b2_v = load_vec(nc.scalar, b2_h, "b2")
                gain_c = [gain_v[:, c : c + 1] for c in range(NCH)]
                b0_c = [b0_v[:, c : c + 1] for c in range(NCH)]
                b1_c = [b1_v[:, c : c + 1] for c in range(NCH)]
                b2_c = [b2_v[:, c : c + 1] for c in range(NCH)]

            # ---- activation image tiles; image 0/1 memsets first --------
            act_img = [[None] * N_PER for _ in range(NPAIR)]
            for n in range(N_PER):
                for q in range(NPAIR):
                    act_img[q][n] = persist.tile(
                        [128, 2, ACT_IMG], FP8, tag=f"act{q}_{n}", name=f"act{q}_{n}"
                    )
            for n in (0, 1):
                for q in range(NPAIR):
                    nc.gpsimd.memset(act_img[q][n], 0.0)

            # ---- HAM warm-up: junk transposes keep PE busy pre-conv ------
            junk_ps = psum_tr.tile([128, 2, 128], BF16, tag="ptr", name="junk")
            for _ in range(130):
                nc.tensor.transpose(junk_ps[:, 0, :], ident, ident)

            # ---- weight + x DMAs in hw-queue need order ------------------
            w_flat = w_ap.rearrange("o i a b -> o (i a b)")
            w_tiles = [None] * NCH
            xs_tiles = [
                persist.tile([128, N_PER, H, W], FP32, tag=f"xs{c}", name=f"xs{c}")
                for c in range(NCH)
            ]
            xr = x_ap.rearrange("n c h w -> c n h w")

            def x_dma(eng, c, n0, n1):
                eng.dma_start(
                    out=xs_tiles[c][:, n0:n1],
                    in_=xr[c * 128 : (c + 1) * 128, n0:n1],
                )

            def w_dma_chunk(m):
                w_tiles[m] = scratch.tile(
                    [128, KFAN], FP32, tag="wtile", name=f"wt{m}"
                )
                nc.sync.dma_start(
                    out=w_tiles[m],
                    in_=w_flat[m * 128 : (m + 1) * 128, :],
                )

            w_tiles[0] = scratch.tile([128, KFAN], FP32, tag="wtile", name="wt0")
            for j in range(TAPS):
                eng = nc.sync if j % 2 == 0 else nc.scalar
                eng.dma_start(
                    out=w_tiles[0][:, j * 512 : (j + 1) * 512],
                    in_=w_flat[0:128, j * 512 : (j + 1) * 512],
                )
            x_dma(nc.sync, 0, 0, 1)
            x_dma(nc.sync, 1, 0, 1)
            x_dma(nc.sync, 2, 0, 2)   # pair-1 acts for images 0-1
            x_dma(nc.sync, 3, 0, 2)
            x_dma(nc.sync, 0, 1, 2)   # pair-0 acts for image 1
            x_dma(nc.sync, 1, 1, 2)
            x_dma(nc.sync, 0, 2, 4)   # image 2-3 pair-0
            x_dma(nc.sync, 1, 2, 4)
            w_dma_chunk(1)
            x_dma(nc.sync, 2, 2, 4)   # image 2-3 pair-1
            x_dma(nc.sync, 3, 2, 4)
            w_dma_chunk(2)
            w_dma_chunk(3)

            # ---- act sign ------------------------------------------------
            def xsign(n, c):
                dst = act_img[c // 2][n][:, c % 2, : HP * WP].rearrange(
                    "p (h w) -> p h w", w=WP
                )[:, 1 : 1 + H, 1 : 1 + W]
                nc.scalar.activation(
                    out=dst,
                    in_=xs_tiles[c][:, n],
                    func=mybir.ActivationFunctionType.Sign,
                    bias=0.0 if specialized else b0_c[c],
                    scale=BETA,
                )

            # lhsT : [128(cin), tap, pair, half, cout] fp8 DoubleRow weights
            lhsT = persist.tile(
                [128, TAPS, NPAIR, 2, C], FP8, tag="lhsT", name="lhsT"
            )
            alphabar = {}   # per cout chunk [128,1]: (2*)alpha*sf*gain
            wsigns = {}
            mvs = {}

            def weight_prep_a(m):
                """stats -> negmean: the critical path to the signs"""
                wt = w_tiles[m]
                st = stats.tile([128, TAPS, 6], FP32, tag="bnst", name="bnst")
                wt3 = wt.rearrange("p (a b) -> p a b", b=512)
                for sg in range(TAPS):
                    nc.vector.bn_stats(out=st[:, sg, :], in_=wt3[:, sg, :])
                mv = stats.tile([128, 2], FP32, tag="bnagg", name="bnagg")
                nc.vector.bn_aggr(out=mv, in_=st)

                negmean = stats.tile([128, 1], FP32, tag="negmean", name="negmean")
                nc.vector.tensor_scalar_mul(out=negmean, in0=mv[:, 0:1], scalar1=-1.0)
                mvs[m] = (mv, negmean)

            def weight_sign(m, q):
                """sign(w - mean) -> bf16 (+-1) for pair q's two cin blocks"""
                wt = w_tiles[m]
                _, negmean = mvs[m]
                if q == 0:
                    ws = scratch.tile([128, KFAN], BF16, tag="wsign", name="wsign")
                    wsigns[m] = ws
                ws = wsigns[m]
                for h2 in range(2):
                    b = 2 * q + h2
                    nc.scalar.activation(
                        out=ws[:, b * 1152 : (b + 1) * 1152],
                        in_=wt[:, b * 1152 : (b + 1) * 1152],
                        func=mybir.ActivationFunctionType.Sign,
                        bias=negmean,
                    )

            def weight_transpose(m, q):
                """per tap: 2 block transposes -> 1 fp8 cast (pair q)"""
                ws3 = wsigns[m].rearrange("p (i t) -> p i t", t=TAPS)
                for t in range(TAPS):
                    ps = psum_tr.tile(
                        [128, 2, 128], BF16, tag="ptr", name="ptr"
                    )
                    for h2 in range(2):
                        b = 2 * q + h2
                        nc.tensor.transpose(
                            ps[:, h2, :],
                            ws3[:, b * 128 : (b + 1) * 128, t],
                            ident,
                        )
                    nc.vector.tensor_copy(
                        out=lhsT[:, t, q, :, m * 128 : (m + 1) * 128],
                        in_=ps,
                    )

            def weight_prep_c(m):
                """1/(std+eps), sum|w-mean| -> alphabar; off critical path"""
                wt = w_tiles[m]
                mv, negmean = mvs[m]
                stdeps = stats.tile([128, 1], FP32, tag="stdeps", name="stdeps")
                nc.scalar.activation(
                    out=stdeps, in_=mv[:, 1:2], func=mybir.ActivationFunctionType.Sqrt
                )
                nc.vector.tensor_scalar_add(out=stdeps, in0=stdeps, scalar1=EPS)
                inv = stats.tile([128, 1], FP32, tag="inv", name="inv")
                nc.vector.reciprocal(out=inv, in_=stdeps)

                sumabs = stats.tile([128, NCH], FP32, tag="sumabs", name="sumabs")
                for b in range(NCH):
                    nc.scalar.activation(
                        out=wt[:, b * 1152 : (b + 1) * 1152],
                        in_=wt[:, b * 1152 : (b + 1) * 1152],
                        func=mybir.ActivationFunctionType.Abs,
                        bias=negmean,
                        accum_out=sumabs[:, b : b + 1],
                    )
                sumabs1 = stats.tile([128, 1], FP32, tag="sumabs1", name="sumabs1")
                nc.vector.tensor_reduce(
                    out=sumabs1, in_=sumabs, axis=mybir.AxisListType.X,
                    op=mybir.AluOpType.add,
                )

                ab = persist.tile(
                    [128, 1], FP32, tag=f"alphabar{m}", name=f"alphabar{m}"
                )
                nc.vector.tensor_tensor(
                    out=ab, in0=sumabs1, in1=inv, op=mybir.AluOpType.mult
                )
                if specialized:
                    # gain == 1
                    nc.vector.tensor_scalar_mul(
                        out=ab, in0=ab, scalar1=ALPHA * WS_SCALE / KFAN
                    )
                else:
                    nc.vector.tensor_tensor(
                        out=ab, in0=ab, in1=gain_c[m], op=mybir.AluOpType.mult
                    )
                    nc.vector.tensor_scalar_mul(
                        out=ab, in0=ab, scalar1=ALPHA * WS_SCALE / KFAN
                    )
                alphabar[m] = ab

            def epilogue(m, n, h2, acc, o=None):
                y0 = h2 * ROWS_PER_TILE
                accv = acc.rearrange("p (h w) -> p h w", w=WP)[:, :, 0:W]
                res = xs_tiles[m][:, n, y0 : y0 + ROWS_PER_TILE, :]
                if specialized:
                    # z = acc*alphabar + residual ; out = prelu(z); biases 0
                    z = epi.tile(
                        [128, ROWS_PER_TILE, W], FP32, tag="z", name="z"
                    )
                    nc.vector.scalar_tensor_tensor(
                        out=z, in0=accv, scalar=alphabar[m], in1=res,
                        op0=mybir.AluOpType.mult, op1=mybir.AluOpType.add,
                    )
                    nc.scalar.activation(
                        out=o[:, y0 : y0 + ROWS_PER_TILE, :], in_=z,
                        func=mybir.ActivationFunctionType.Prelu,
                        alpha=pa_c[m],
                    )
                    if h2 == NSPAT - 1:
                        # one batched store for the whole (chunk, image)
                        nc.sync.dma_start(
                            out=out_ap[n, m * 128 : (m + 1) * 128, :, :],
                            in_=o,
                        )
                    return
                # z = acc*alphabar + residual   (prelu input minus b1)
                z = epi.tile(
                    [128, ROWS_PER_TILE, W], FP32, tag="z", name="z"
                )
                nc.vector.scalar_tensor_tensor(
                    out=z, in0=accv, scalar=alphabar[m], in1=res,
                    op0=mybir.AluOpType.mult, op1=mybir.AluOpType.add,
                )
                # r = relu(z + b1) on ACT
                r = epi.tile(
                    [128, ROWS_PER_TILE, W], FP32, tag="r", name="r"
                )
                nc.scalar.activation(
                    out=r, in_=z,
                    func=mybir.ActivationFunctionType.Relu,
                    bias=b1_c[m],
                )
                # zz = a*z + (a*b1 + b2) ; out = (1-a)*r + zz
                zz = epi.tile(
                    [128, ROWS_PER_TILE, W], FP32, tag="zz", name="zz"
                )
                nc.scalar.activation(
                    out=zz, in_=z,
                    func=mybir.ActivationFunctionType.Identity,
                    scale=pa_c[m], bias=ab1b2[m],
                )
                nc.vector.scalar_tensor_tensor(
                    out=zz, in0=r, scalar=one_minus_a[m], in1=zz,
                    op0=mybir.AluOpType.mult, op1=mybir.AluOpType.add,
                )
                nc.sync.dma_start(
                    out=out_ap[
                        n, m * 128 : (m + 1) * 128,
                        y0 : y0 + ROWS_PER_TILE, :,
                    ],
                    in_=zz,
                )

            def conv_rounds(m, group, accs, q):
                """emit the 9 accumulation rounds of pair q for a group"""
                for t in range(TAPS):
                    dy, dx = t // 3, t % 3
                    wslice = lhsT[:, t, q, :, m * 128 : (m + 1) * 128]
                    for j, (n, h2) in enumerate(group):
                        base = (h2 * ROWS_PER_TILE + dy) * WP + dx
                        rhs = act_img[q][n][:, :, base : base + NFREE]
                        nc.tensor.matmul(
                            accs[j],
                            wslice,
                            rhs,
                            start=(q == 0 and t == 0),
                            stop=(q == NPAIR - 1 and t == TAPS - 1),
                            perf_mode=mybir.MatmulPerfMode.DoubleRow,
                        )

            def conv_open(m, group):
                return [
                    psum_mm.tile([128, NFREE], FP32, tag="acc", name="acc")
                    for _ in group
                ]

            def group_out_tile():
                if not specialized:
                    return None
                return epi.tile([128, H, W], FP32, tag="o", name="o")

            def conv_group(m, group):
                accs = conv_open(m, group)
                for q in range(NPAIR):
                    conv_rounds(m, group, accs, q)
                o = group_out_tile()
                for j, (n, h2) in enumerate(group):
                    epilogue(m, n, h2, accs[j], o)

            # ---- chunk 0 startup pipeline --------------------------------
            weight_prep_a(0)
            weight_sign(0, 0)          # blocks 0,1
            weight_transpose(0, 0)     # taps for pair 0
            xsign(0, 0)
            xsign(0, 1)
            weight_sign(0, 1)          # blocks 2,3

            if not specialized:
                # derived per-channel epilogue constants (emitted after the
                # startup-critical DVE work so they can't HOL-block the FIFO)
                oma_v = persist.tile([128, NCH], FP32, tag="oma")
                nc.vector.tensor_scalar(
                    out=oma_v, in0=pa_v, scalar1=-1.0, scalar2=1.0,
                    op0=mybir.AluOpType.mult, op1=mybir.AluOpType.add,
                )
                one_minus_a = [oma_v[:, c : c + 1] for c in range(NCH)]
                ab_v = persist.tile([128, NCH], FP32, tag="ab1b2")
                for c in range(NCH):
                    nc.vector.scalar_tensor_tensor(
                        out=ab_v[:, c : c + 1], in0=b1_c[c], scalar=pa_c[c],
                        in1=b2_c[c],
                        op0=mybir.AluOpType.mult, op1=mybir.AluOpType.add,
                    )
                ab1b2 = [ab_v[:, c : c + 1] for c in range(NCH)]

            g0 = [(0, 0), (0, 1)]
            g0_accs = conv_open(0, g0)
            conv_rounds(0, g0, g0_accs, 0)
            xsign(0, 2)
            xsign(0, 3)
            xsign(1, 0)
            xsign(1, 1)
            weight_transpose(0, 1)     # taps for pair 1
            weight_prep_c(0)
            conv_rounds(0, g0, g0_accs, 1)
            g0_o = group_out_tile()
            for j, (n, h2) in enumerate(g0):
                epilogue(0, n, h2, g0_accs[j], g0_o)
            xsign(1, 2)
            xsign(1, 3)
            conv_group(0, [(1, 0), (1, 1)])
            for q in range(NPAIR):
                nc.gpsimd.memset(act_img[q][2], 0.0)
            for c in range(NCH):
                xsign(2, c)
            conv_group(0, [(2, 0), (2, 1)])
            for q in range(NPAIR):
                nc.gpsimd.memset(act_img[q][3], 0.0)
            for c in range(NCH):
                xsign(3, c)
            weight_prep_a(1)
            weight_sign(1, 0)
            weight_sign(1, 1)
            weight_transpose(1, 0)
            weight_transpose(1, 1)
            conv_group(0, [(3, 0), (3, 1)])
            weight_prep_c(1)

            # ---- main loop: conv(m) interleaved with prep(m+1) -----------
            for m in range(1, NCH):
                conv_group(m, [(0, 0), (0, 1)])
                if m + 1 < NCH:
                    weight_prep_a(m + 1)
                    weight_sign(m + 1, 0)
                    weight_sign(m + 1, 1)
                conv_group(m, [(1, 0), (1, 1)])
                if m + 1 < NCH:
                    weight_transpose(m + 1, 0)
                conv_group(m, [(2, 0), (2, 1)])
                if m + 1 < NCH:
                    weight_transpose(m + 1, 1)
                conv_group(m, [(3, 0), (3, 1)])
                if m + 1 < NCH:
                    weight_prep_c(m + 1)

    nc.finalize()
    return nc


_NC_CACHE = {}


def _get_program(specialized=True):
    if specialized not in _NC_CACHE:
        _NC_CACHE[specialized] = build_program(specialized)
    return _NC_CACHE[specialized]


def kernel(**inputs):
    from concourse.bass_utils import run_bass_kernel_spmd

    x = np.ascontiguousarray(np.asarray(inputs["x"], dtype=np.float32))
    shared = {
        name: np.ascontiguousarray(np.asarray(inputs[name], dtype=np.float32))
        for name in (
            "conv_weight", "gain", "move0_bias", "move1_bias", "prelu_a",
            "move2_bias",
        )
    }
    specialized = bool(
        np.all(shared["move0_bias"] == 0.0)
        and np.all(shared["move1_bias"] == 0.0)
        and np.all(shared["move2_bias"] == 0.0)
        and np.all(shared["gain"] == 1.0)
    )
    nc = _get_program(specialized)
    in_maps = [
        {"x": x[i * N_PER : (i + 1) * N_PER], **shared} for i in range(N_CORES)
    ]
    res = run_bass_kernel_spmd(nc, in_maps, core_ids=list(range(N_CORES)))
    return np.concatenate([r["out"] for r in res.results], axis=0)
